# revision 10
# baseline (speedup 1.0000x reference)
"""Trainium2 Bass kernel for the CustomCRFLoss problem.

Strategy (pure data parallel, one sample per NeuronCore, 8 cores):

The reference collapses to:
    s_0[j] = colsum(unary)[j],  s_{t+1} = s_t - M s_t,  answer = 1^T s_5
with M[j,w] = M1[j,w] + M2[j,w]:
    M1[j,w] = sum_i k(x_ij, x_iw)   (spatial, row pairs)
    M2[j,w] = sum_i k(x_ij, x_wj)   (bilateral, column pairs)
and k(a,b) = exp(-||a-b||^2/2) = exp(-|a|^2/2) exp(-|b|^2/2) exp(a.b).

Both terms use ONE degree-2 Taylor feature tile TT[p,m,j] = phi_m(x at row
p, col j) (10 monomial features x^alpha/sqrt(alpha!) * exp(-r/2), bf16):
    MT1 = sum_m T_m^T T_m                      (10 PSUM-accumulated matmuls)
    MT2[w,j] = sum_m T_m[w,j] * c_m[j],  c_m[j] = sum_i T_m[i,j]
The partition-broadcast column sums come free from CC = J @ TT with J the
all-ones matrix (3 matmuls), so MT2 is 3 elementwise muls and a small add
tree.  No exps beyond the single E0, no per-column Gram matmuls.

Slot order is chosen so CC chunk 0 = [E0, diag] depends only on E0 (the
earliest features), and the unary runs on the otherwise idle Pool engine.
Tail: 4 iterations of (matvec + subtract); the 5th iteration is folded into
a host-side dot: answer = <1 - rowsum(MT), s_4>, with rowsum(MT) from the
Activation engine's accumulate output.  The kernel ships [s_4 | g] per core.

Emulated accuracy vs the f64 reference: rel err ~1e-3 (gate 2e-2).
"""

import math

import numpy as np

import concourse.bass as bass
import concourse.tile as tile
from concourse import mybir
from concourse.bass_utils import run_bass_kernel_spmd
from concourse.tile import add_dep_helper

H = W = 128
NB = 8  # batch / cores
NMON = 10

F32 = mybir.dt.float32
BF16 = mybir.dt.bfloat16
AF = mybir.ActivationFunctionType
ALU = mybir.AluOpType
AX = mybir.AxisListType

ISQ2 = 1.0 / math.sqrt(2.0)


def _bcast(ap, wid):
    """[P,128] AP -> [P,wid,128] with a step-0 middle dim."""
    return bass.AP(
        tensor=ap.tensor,
        offset=ap.offset,
        ap=[list(ap.ap[0]), [0, wid], list(ap.ap[1])],
    )


def build_kernel():
    nc = bass.Bass()
    im_d = nc.dram_tensor("imb", (H, 3, W), BF16, kind="ExternalInput")
    lg_d = nc.dram_tensor("lg", (H, 2, W), F32, kind="ExternalInput")
    lb_d = nc.dram_tensor("lb", (H, W), F32, kind="ExternalInput")
    out_d = nc.dram_tensor("out", (H, 2), F32, kind="ExternalOutput")

    with tile.TileContext(nc) as tc:
        with (
            tc.tile_pool(name="sb", bufs=1) as sb,
            tc.tile_pool(name="qp", bufs=3) as qpool,
            tc.tile_pool(name="pm", bufs=1, space="PSUM") as pm,
            tc.tile_pool(name="pc", bufs=1, space="PSUM") as pc,
            tc.tile_pool(name="ps", bufs=2, space="PSUM") as ps,
        ):
            # ---------------- input DMAs (dense, partition-major) ----------
            imtile = sb.tile([H, 3, W], BF16)
            nc.sync.dma_start(out=imtile, in_=im_d[:])
            lg = sb.tile([H, 2, W], F32)
            nc.sync.dma_start(out=lg, in_=lg_d[:])
            lb = sb.tile([H, W], F32)
            nc.sync.dma_start(out=lb, in_=lb_d[:])

            # ---------------- constants ----------------
            ones_mat = sb.tile([H, W], BF16)
            nc.gpsimd.memset(ones_mat, 1.0)
            ones_col = sb.tile([H, 1], F32)
            nc.gpsimd.memset(ones_col, 1.0)

            # ------------- feature build (DVE critical path) ---------------
            # sq = x^2 ; r = sum_c sq_c ; E0 = exp(-r/2) via ACT scale
            sq = sb.tile([H, 3, W], BF16)
            nc.vector.tensor_mul(out=sq, in0=imtile, in1=imtile)
            a1 = sb.tile([H, W], BF16)
            nc.vector.tensor_add(out=a1, in0=sq[:, 0, :], in1=sq[:, 1, :])
            rs = sb.tile([H, W], BF16)
            nc.vector.tensor_add(out=rs, in0=a1, in1=sq[:, 2, :])
            # sqh = x^2/sqrt(2) for the diag features (off critical path)
            sqh = sb.tile([H, 3, W], BF16)
            nc.vector.tensor_scalar_mul(out=sqh, in0=sq, scalar1=ISQ2)

            # TT slots: 0=E0, 1:4=x_c^2/sqrt2*E0 (diag), 4:7=x_c*E0 (deg1),
            # 7=x1x2E0, 8=x1x3E0, 9=x2x3E0
            TT = sb.tile([H, NMON, W], BF16)
            _e0 = nc.scalar.activation(
                out=TT[:, 0, :], in_=rs, func=AF.Exp, scale=-0.5
            )
            E0 = TT[:, 0, :]
            nc.vector.tensor_mul(out=TT[:, 1:4, :], in0=sqh, in1=_bcast(E0, 3))
            nc.vector.tensor_mul(out=TT[:, 4:7, :], in0=imtile, in1=_bcast(E0, 3))
            T1 = TT[:, 4, :]
            nc.vector.tensor_mul(
                out=TT[:, 7:9, :], in0=_bcast(T1, 2), in1=imtile[:, 1:3, :]
            )
            nc.vector.tensor_mul(
                out=TT[:, 9:10, :], in0=TT[:, 5:6, :], in1=imtile[:, 2:3, :]
            )

            # ---------------- unary (Pool + ACT, off critical path) --------
            dd = sb.tile([H, W], F32)
            nc.gpsimd.tensor_sub(out=dd, in0=lg[:, 1, :], in1=lg[:, 0, :])
            ed = sb.tile([H, W], F32)
            _ed = nc.scalar.activation(out=ed, in_=dd, func=AF.Exp)
            # keep the ACT queue free for E0 (the global gate) first
            add_dep_helper(_ed.ins, _e0.ins, False, "E0 before unary exp")
            sp = sb.tile([H, W], F32)
            nc.scalar.activation(out=sp, in_=ed, func=AF.Ln, bias=1.0)
            tl = sb.tile([H, W], F32)
            nc.gpsimd.tensor_mul(out=tl, in0=lb, in1=dd)
            u = sb.tile([H, W], F32)
            nc.gpsimd.tensor_sub(out=u, in0=sp, in1=tl)

            # ---------------- PE: CC chunks first, then MT1 ----------------
            mt1 = pm.tile([H, W], F32)
            nc.tensor.matmul(
                mt1, lhsT=TT[:, 0, :], rhs=TT[:, 0, :], start=True, stop=False
            )
            cc0 = pc.tile([H, 4 * W], F32, tag="cc0")
            nc.tensor.matmul(
                cc0, lhsT=ones_mat, rhs=TT[:, 0:4, :], start=True, stop=True
            )
            cc1 = pc.tile([H, 4 * W], F32, tag="cc1")
            nc.tensor.matmul(
                cc1, lhsT=ones_mat, rhs=TT[:, 4:8, :], start=True, stop=True
            )
            cc2 = pc.tile([H, 2 * W], F32, tag="cc2")
            nc.tensor.matmul(
                cc2, lhsT=ones_mat, rhs=TT[:, 8:10, :], start=True, stop=True
            )
            for m in range(1, NMON):
                nc.tensor.matmul(
                    mt1, lhsT=TT[:, m, :], rhs=TT[:, m, :],
                    start=False, stop=(m == NMON - 1),
                )
            q0p = ps.tile([H, 1], F32, tag="s")
            nc.tensor.matmul(q0p, lhsT=u, rhs=ones_col, start=True, stop=True)

            # -------- products (DVE, PSUM-direct) + chunk-local tree -------
            # last chunk goes through an ACT copy so its product is 2x
            ccs2 = sb.tile([H, 2, W], BF16)
            nc.scalar.activation(out=ccs2, in_=cc2, func=AF.Copy)

            P = sb.tile([H, NMON, W], BF16)
            nc.vector.tensor_mul(out=P[:, 0:4, :], in0=TT[:, 0:4, :], in1=cc0)
            # chunk-0 tree branch on the otherwise idle Pool engine
            r1a = sb.tile([H, 2, W], BF16)
            nc.gpsimd.tensor_add(out=r1a, in0=P[:, 0:2, :], in1=P[:, 2:4, :])
            r1s = sb.tile([H, W], BF16)
            nc.gpsimd.tensor_add(out=r1s, in0=r1a[:, 0, :], in1=r1a[:, 1, :])
            nc.vector.tensor_mul(out=P[:, 4:8, :], in0=TT[:, 4:8, :], in1=cc1)
            s1a = sb.tile([H, 2, W], BF16)
            nc.vector.tensor_add(out=s1a, in0=P[:, 4:6, :], in1=P[:, 6:8, :])
            s1s = sb.tile([H, W], BF16)
            nc.vector.tensor_add(out=s1s, in0=s1a[:, 0, :], in1=s1a[:, 1, :])
            nc.vector.tensor_mul(out=P[:, 8:10, :], in0=TT[:, 8:10, :], in1=ccs2)
            t3 = sb.tile([H, W], BF16)
            nc.vector.tensor_add(out=t3, in0=P[:, 8, :], in1=P[:, 9, :])
            f1 = sb.tile([H, W], BF16)
            nc.vector.tensor_add(out=f1, in0=r1s, in1=s1s)
            mt2b = sb.tile([H, W], BF16)
            nc.vector.tensor_add(out=mt2b, in0=f1, in1=t3)

            # ---------------- M, iterations ----------------
            qcur = qpool.tile([H, 1], F32, tag="q")
            nc.vector.tensor_copy(out=qcur, in_=q0p)
            MTs = sb.tile([H, W], F32)
            nc.vector.tensor_add(out=MTs, in0=mt1, in1=mt2b)

            # out layout: vout[:,0]=s_4, vout[:,1]=g=rowsum(MT)
            vout = sb.tile([H, 2], F32)
            gdump = sb.tile([H, W], BF16)
            nc.scalar.activation(
                out=gdump, in_=MTs, func=AF.Copy, accum_out=vout[:, 1:2]
            )

            for it in range(4):
                yp = ps.tile([H, 1], F32, tag="s")
                nc.tensor.matmul(yp, lhsT=MTs, rhs=qcur, start=True, stop=True)
                if it < 3:
                    qn = qpool.tile([H, 1], F32, tag="q")
                    nc.vector.tensor_sub(out=qn, in0=qcur, in1=yp)
                    qcur = qn
                else:
                    nc.vector.tensor_sub(out=vout[:, 0:1], in0=qcur, in1=yp)

            nc.sync.dma_start(out=out_d[:], in_=vout)

    return nc


def _split_excess_waits(nc, max_waits=1, max_updates=1):
    """The walrus build in this container rejects instructions whose Events
    carry more than one semaphore wait (ISA Events has a single wait slot).
    Tile's sem assignment can attach several.  Split the extras onto
    same-engine NoOps placed immediately before (waits) / after (updates)
    the instruction; sequencers execute in order, so semantics are kept."""
    for fn in nc.m.functions:
        for bb in fn.blocks:
            ins = bb.instructions
            out = []
            changed = False
            for inst in ins:
                si = inst.sync_info
                if si is None:
                    out.append(inst)
                    continue
                waits = list(si.on_wait or [])
                updates = list(si.on_update or [])
                if len(waits) <= max_waits and len(updates) <= max_updates:
                    out.append(inst)
                    continue
                changed = True
                pre, post = [], []
                if len(waits) > max_waits:
                    for k, wt in enumerate(waits[:-max_waits]):
                        pre.append(
                            mybir.InstNoOp(
                                name=f"{inst.name}-w{k}",
                                engine=inst.engine,
                                bass_nofuse=True,
                                sync_info=mybir.SyncInfo(on_wait=[wt], on_update=[]),
                            )
                        )
                    waits = waits[-max_waits:]
                if len(updates) > max_updates:
                    for k, up in enumerate(updates[max_updates:]):
                        post.append(
                            mybir.InstNoOp(
                                name=f"{inst.name}-u{k}",
                                engine=inst.engine,
                                bass_nofuse=True,
                                sync_info=mybir.SyncInfo(on_wait=[], on_update=[up]),
                            )
                        )
                    updates = updates[:max_updates]
                inst.sync_info = mybir.SyncInfo(on_wait=waits, on_update=updates)
                out.extend(pre)
                out.append(inst)
                out.extend(post)
            if changed:
                bb.instructions = out
    return nc


_NC_CACHE = None


def kernel(logits, labels, images):
    global _NC_CACHE
    if _NC_CACHE is None:
        _NC_CACHE = _split_excess_waits(build_kernel())
    nc = _NC_CACHE

    import ml_dtypes

    logits = np.asarray(logits, dtype=np.float32)
    labels_f = np.asarray(labels).astype(np.float32)
    images = np.asarray(images, dtype=np.float32)
    imc = images - 0.5
    # partition-major layouts: [H, C, W]
    im_b = np.ascontiguousarray(np.swapaxes(imc, 1, 2).astype(ml_dtypes.bfloat16))
    lg_t = np.ascontiguousarray(np.swapaxes(logits, 1, 2))

    in_maps = [
        {
            "imb": im_b[b],
            "lg": lg_t[b],
            "lb": np.ascontiguousarray(labels_f[b]),
        }
        for b in range(NB)
    ]
    res = run_bass_kernel_spmd(nc, in_maps, core_ids=list(range(NB)))
    tot = 0.0
    for b in range(NB):
        o = res.results[b]["out"].astype(np.float64)
        s4, g = o[:, 0], o[:, 1]
        # answer_b = 1^T s_5 = sum(s_4) - <g, s_4>
        tot += s4.sum() - float(g @ s4)
    return np.float32(tot / (NB * H * W))


# revision 12
# speedup vs baseline: 1.0168x; 1.0168x over previous
"""Trainium2 Bass kernel for the CustomCRFLoss problem.

Strategy (pure data parallel, one sample per NeuronCore, 8 cores):

The reference collapses to:
    s_0[j] = colsum(unary)[j],  s_{t+1} = s_t - M s_t,  answer = 1^T s_5
with M[j,w] = M1[j,w] + M2[j,w]:
    M1[j,w] = sum_i k(x_ij, x_iw)   (spatial, row pairs)
    M2[j,w] = sum_i k(x_ij, x_wj)   (bilateral, column pairs)
and k(a,b) = exp(-||a-b||^2/2) = exp(-|a|^2/2) exp(-|b|^2/2) exp(a.b).

Both terms use ONE degree-2 Taylor feature tile TT[p,m,j] = phi_m(x at row
p, col j) (10 monomial features x^alpha/sqrt(alpha!) * exp(-r/2), bf16):
    MT1 = sum_m T_m^T T_m                      (10 PSUM-accumulated matmuls)
    MT2[w,j] = sum_m T_m[w,j] * c_m[j],  c_m[j] = sum_i T_m[i,j]
The partition-broadcast column sums come free from CC = J @ TT with J the
all-ones matrix (3 matmuls), so MT2 is 3 elementwise muls and a small add
tree.  No exps beyond the single E0, no per-column Gram matmuls.

Slot order is chosen so CC chunk 0 = [E0, diag] depends only on E0 (the
earliest features), and the unary runs on the otherwise idle Pool engine.
Tail: 4 iterations of (matvec + subtract); the 5th iteration is folded into
a host-side dot: answer = <1 - rowsum(MT), s_4>, with rowsum(MT) from the
Activation engine's accumulate output.  The kernel ships [s_4 | g] per core.

Emulated accuracy vs the f64 reference: rel err ~1e-3 (gate 2e-2).
"""

import math

import numpy as np

import concourse.bass as bass
import concourse.tile as tile
from concourse import mybir
from concourse.bass_utils import run_bass_kernel_spmd
from concourse.tile import add_dep_helper

H = W = 128
NB = 8  # batch / cores
NMON = 10

F32 = mybir.dt.float32
BF16 = mybir.dt.bfloat16
AF = mybir.ActivationFunctionType
ALU = mybir.AluOpType
AX = mybir.AxisListType

ISQ2 = 1.0 / math.sqrt(2.0)


def _bcast(ap, wid):
    """[P,128] AP -> [P,wid,128] with a step-0 middle dim."""
    return bass.AP(
        tensor=ap.tensor,
        offset=ap.offset,
        ap=[list(ap.ap[0]), [0, wid], list(ap.ap[1])],
    )


def build_kernel():
    nc = bass.Bass()
    im_d = nc.dram_tensor("imb", (H, 3, W), BF16, kind="ExternalInput")
    lg_d = nc.dram_tensor("lg", (H, 2, W), F32, kind="ExternalInput")
    lb_d = nc.dram_tensor("lb", (H, W), F32, kind="ExternalInput")
    out_d = nc.dram_tensor("out", (H, 2), F32, kind="ExternalOutput")

    with tile.TileContext(nc) as tc:
        with (
            tc.tile_pool(name="sb", bufs=1) as sb,
            tc.tile_pool(name="qp", bufs=3) as qpool,
            tc.tile_pool(name="pm", bufs=1, space="PSUM") as pm,
            tc.tile_pool(name="pc", bufs=1, space="PSUM") as pc,
            tc.tile_pool(name="ps", bufs=2, space="PSUM") as ps,
        ):
            # ---------------- input DMAs (dense, partition-major) ----------
            imtile = sb.tile([H, 3, W], BF16)
            nc.sync.dma_start(out=imtile, in_=im_d[:])
            lg = sb.tile([H, 2, W], F32)
            nc.sync.dma_start(out=lg, in_=lg_d[:])
            lb = sb.tile([H, W], F32)
            nc.sync.dma_start(out=lb, in_=lb_d[:])

            # ---------------- constants ----------------
            ones_mat = sb.tile([H, W], BF16)
            nc.gpsimd.memset(ones_mat, 1.0)
            ones_col = sb.tile([H, 1], F32)
            nc.gpsimd.memset(ones_col, 1.0)

            # ------------- feature build (DVE critical path) ---------------
            # sq = x^2 ; r = sum_c sq_c ; E0 = exp(-r/2) via ACT scale
            sq = sb.tile([H, 3, W], BF16)
            nc.vector.tensor_mul(out=sq, in0=imtile, in1=imtile)
            a1 = sb.tile([H, W], BF16)
            nc.vector.tensor_add(out=a1, in0=sq[:, 0, :], in1=sq[:, 1, :])
            rs = sb.tile([H, W], BF16)
            nc.vector.tensor_add(out=rs, in0=a1, in1=sq[:, 2, :])
            # sqh = x^2/sqrt(2) for the diag features (off critical path)
            sqh = sb.tile([H, 3, W], BF16)
            nc.vector.tensor_scalar_mul(out=sqh, in0=sq, scalar1=ISQ2)

            # TT slots: 0=E0, 1:4=x_c^2/sqrt2*E0 (diag), 4:7=x_c*E0 (deg1),
            # 7=x1x2E0, 8=x1x3E0, 9=x2x3E0
            TT = sb.tile([H, NMON, W], BF16)
            _e0 = nc.scalar.activation(
                out=TT[:, 0, :], in_=rs, func=AF.Exp, scale=-0.5
            )
            E0 = TT[:, 0, :]
            nc.vector.tensor_mul(out=TT[:, 1:4, :], in0=sqh, in1=_bcast(E0, 3))
            nc.vector.tensor_mul(out=TT[:, 4:7, :], in0=imtile, in1=_bcast(E0, 3))
            T1 = TT[:, 4, :]
            nc.vector.tensor_mul(
                out=TT[:, 7:9, :], in0=_bcast(T1, 2), in1=imtile[:, 1:3, :]
            )
            nc.vector.tensor_mul(
                out=TT[:, 9:10, :], in0=TT[:, 5:6, :], in1=imtile[:, 2:3, :]
            )

            # ---------------- unary (Pool + ACT, off critical path) --------
            dd = sb.tile([H, W], F32)
            nc.gpsimd.tensor_sub(out=dd, in0=lg[:, 1, :], in1=lg[:, 0, :])
            ed = sb.tile([H, W], F32)
            _ed = nc.scalar.activation(out=ed, in_=dd, func=AF.Exp)
            # keep the ACT queue free for E0 (the global gate) first
            add_dep_helper(_ed.ins, _e0.ins, False, "E0 before unary exp")
            sp = sb.tile([H, W], F32)
            nc.scalar.activation(out=sp, in_=ed, func=AF.Ln, bias=1.0)
            tl = sb.tile([H, W], F32)
            nc.gpsimd.tensor_mul(out=tl, in0=lb, in1=dd)
            u = sb.tile([H, W], F32)
            nc.gpsimd.tensor_sub(out=u, in0=sp, in1=tl)

            # ---------------- PE: CC chunks first, then MT1 ----------------
            mt1 = pm.tile([H, W], F32)
            nc.tensor.matmul(
                mt1, lhsT=TT[:, 0, :], rhs=TT[:, 0, :], start=True, stop=False
            )
            cc0 = pc.tile([H, 4 * W], F32, tag="cc0")
            nc.tensor.matmul(
                cc0, lhsT=ones_mat, rhs=TT[:, 0:4, :], start=True, stop=True
            )
            cc1 = pc.tile([H, 4 * W], F32, tag="cc1")
            nc.tensor.matmul(
                cc1, lhsT=ones_mat, rhs=TT[:, 4:8, :], start=True, stop=True
            )
            cc2 = pc.tile([H, 2 * W], F32, tag="cc2")
            nc.tensor.matmul(
                cc2, lhsT=ones_mat, rhs=TT[:, 8:10, :], start=True, stop=True
            )
            for m in range(1, NMON):
                nc.tensor.matmul(
                    mt1, lhsT=TT[:, m, :], rhs=TT[:, m, :],
                    start=False, stop=(m == NMON - 1),
                )
            q0p = ps.tile([H, 1], F32, tag="s")
            nc.tensor.matmul(q0p, lhsT=u, rhs=ones_col, start=True, stop=True)

            # -------- products (DVE, PSUM-direct) + chunk-local tree -------
            # last chunk goes through an ACT copy so its product is 2x
            ccs2 = sb.tile([H, 2, W], BF16)
            nc.scalar.activation(out=ccs2, in_=cc2, func=AF.Copy)
            # mt1 PSUM -> SBUF f32 on idle ACT so the final add avoids the
            # PSUM read penalty and lands earlier
            mt1f = sb.tile([H, W], F32)
            nc.scalar.activation(out=mt1f, in_=mt1, func=AF.Copy)

            P = sb.tile([H, NMON, W], BF16)
            nc.vector.tensor_mul(out=P[:, 0:4, :], in0=TT[:, 0:4, :], in1=cc0)
            # chunk-0 tree branch on the otherwise idle Pool engine
            r1a = sb.tile([H, 2, W], BF16)
            nc.gpsimd.tensor_add(out=r1a, in0=P[:, 0:2, :], in1=P[:, 2:4, :])
            r1s = sb.tile([H, W], BF16)
            nc.gpsimd.tensor_add(out=r1s, in0=r1a[:, 0, :], in1=r1a[:, 1, :])
            nc.vector.tensor_mul(out=P[:, 4:8, :], in0=TT[:, 4:8, :], in1=cc1)
            s1a = sb.tile([H, 2, W], BF16)
            nc.vector.tensor_add(out=s1a, in0=P[:, 4:6, :], in1=P[:, 6:8, :])
            s1s = sb.tile([H, W], BF16)
            nc.vector.tensor_add(out=s1s, in0=s1a[:, 0, :], in1=s1a[:, 1, :])
            nc.vector.tensor_mul(out=P[:, 8:10, :], in0=TT[:, 8:10, :], in1=ccs2)
            t3 = sb.tile([H, W], BF16)
            nc.vector.tensor_add(out=t3, in0=P[:, 8, :], in1=P[:, 9, :])
            f1 = sb.tile([H, W], BF16)
            nc.vector.tensor_add(out=f1, in0=r1s, in1=s1s)
            mt2b = sb.tile([H, W], BF16)
            nc.vector.tensor_add(out=mt2b, in0=f1, in1=t3)

            # ---------------- M, iterations ----------------
            qcur = qpool.tile([H, 1], F32, tag="q")
            nc.vector.tensor_copy(out=qcur, in_=q0p)
            MTs = sb.tile([H, W], F32)
            nc.vector.tensor_add(out=MTs, in0=mt1f, in1=mt2b)

            # out layout: vout[:,0]=s_4, vout[:,1]=g=rowsum(MT)
            vout = sb.tile([H, 2], F32)
            gdump = sb.tile([H, W], BF16)
            nc.scalar.activation(
                out=gdump, in_=MTs, func=AF.Copy, accum_out=vout[:, 1:2]
            )

            for it in range(4):
                yp = ps.tile([H, 1], F32, tag="s")
                nc.tensor.matmul(yp, lhsT=MTs, rhs=qcur, start=True, stop=True)
                if it < 3:
                    qn = qpool.tile([H, 1], F32, tag="q")
                    nc.vector.tensor_sub(out=qn, in0=qcur, in1=yp)
                    qcur = qn
                else:
                    nc.vector.tensor_sub(out=vout[:, 0:1], in0=qcur, in1=yp)

            nc.sync.dma_start(out=out_d[:], in_=vout)

    return nc


def _split_excess_waits(nc, max_waits=1, max_updates=1):
    """The walrus build in this container rejects instructions whose Events
    carry more than one semaphore wait (ISA Events has a single wait slot).
    Tile's sem assignment can attach several.  Split the extras onto
    same-engine NoOps placed immediately before (waits) / after (updates)
    the instruction; sequencers execute in order, so semantics are kept."""
    for fn in nc.m.functions:
        for bb in fn.blocks:
            ins = bb.instructions
            out = []
            changed = False
            for inst in ins:
                si = inst.sync_info
                if si is None:
                    out.append(inst)
                    continue
                waits = list(si.on_wait or [])
                updates = list(si.on_update or [])
                if len(waits) <= max_waits and len(updates) <= max_updates:
                    out.append(inst)
                    continue
                changed = True
                pre, post = [], []
                if len(waits) > max_waits:
                    for k, wt in enumerate(waits[:-max_waits]):
                        pre.append(
                            mybir.InstNoOp(
                                name=f"{inst.name}-w{k}",
                                engine=inst.engine,
                                bass_nofuse=True,
                                sync_info=mybir.SyncInfo(on_wait=[wt], on_update=[]),
                            )
                        )
                    waits = waits[-max_waits:]
                if len(updates) > max_updates:
                    for k, up in enumerate(updates[max_updates:]):
                        post.append(
                            mybir.InstNoOp(
                                name=f"{inst.name}-u{k}",
                                engine=inst.engine,
                                bass_nofuse=True,
                                sync_info=mybir.SyncInfo(on_wait=[], on_update=[up]),
                            )
                        )
                    updates = updates[:max_updates]
                inst.sync_info = mybir.SyncInfo(on_wait=waits, on_update=updates)
                out.extend(pre)
                out.append(inst)
                out.extend(post)
            if changed:
                bb.instructions = out
    return nc


_NC_CACHE = None


def kernel(logits, labels, images):
    global _NC_CACHE
    if _NC_CACHE is None:
        _NC_CACHE = _split_excess_waits(build_kernel())
    nc = _NC_CACHE

    import ml_dtypes

    logits = np.asarray(logits, dtype=np.float32)
    labels_f = np.asarray(labels).astype(np.float32)
    images = np.asarray(images, dtype=np.float32)
    imc = images - 0.5
    # partition-major layouts: [H, C, W]
    im_b = np.ascontiguousarray(np.swapaxes(imc, 1, 2).astype(ml_dtypes.bfloat16))
    lg_t = np.ascontiguousarray(np.swapaxes(logits, 1, 2))

    in_maps = [
        {
            "imb": im_b[b],
            "lg": lg_t[b],
            "lb": np.ascontiguousarray(labels_f[b]),
        }
        for b in range(NB)
    ]
    res = run_bass_kernel_spmd(nc, in_maps, core_ids=list(range(NB)))
    tot = 0.0
    for b in range(NB):
        o = res.results[b]["out"].astype(np.float64)
        s4, g = o[:, 0], o[:, 1]
        # answer_b = 1^T s_5 = sum(s_4) - <g, s_4>
        tot += s4.sum() - float(g @ s4)
    return np.float32(tot / (NB * H * W))
